# revision 10
# baseline (speedup 1.0000x reference)
"""Trainium2 Bass kernel for dual (spectral + spatial) multi-head cross-attention.

Reference computation (see problem):
  q, kv: [B=2, S=32, H=24, W=24, C=64], heads=4, head_dim=16, scale=0.25
  spectral: attention over S per (b, pixel, head)
  spatial:  attention over H*W per (b, band, head)
  out = x_spectral + x_spatial

Sharding (8 cores):
  spatial : (b, s) pairs, 64 total -> 8 per core
  spectral: (b, pixel) pairs, 1152 total -> 144 per core (x4 heads = 576 problems)

Device strategy (per core):
  spatial per (b,s):  scores^T[t,p] = K_h Q_h^T via PE (k=16, m=t-chunks of 128,
    serial heads); exp via ACT (scale folded); PV via col-tiled (bank-separated)
    matmuls with an appended ones-column producing softmax denominators for free.
    Outputs un-normalized out^T + denom; host divides + transposes.
  spectral: 4 problems ([32,16] attention) packed block-diagonally into one
    [21,128]x[21,128]->[128,128] matmul with mask rows baked into the operands
    (exp(-30) ~ 0 kills cross-problem terms); PV = [128,128]x[128,68] with
    block-diag V+ones. Host extracts diagonal blocks, divides, scatters.

All matmul inputs are float32r (TF32-like, ~1e-4 rel err, full PE speed at
N>=256). Host pre-rounds DRAM inputs to TF32 so the BIR verifier is satisfied.
"""
import sys

sys.path.insert(0, '/opt/trn_rl_repo')

import numpy as np

B, S, HH, WW, C = 2, 32, 24, 24, 64
NH, HD = 4, 16
SCALE = HD ** -0.5
HW = HH * WW                      # 576
NCORES = 8
NS = (B * S) // NCORES            # 8 spatial (b,s) problems per core
NPX = (B * HW) // NCORES          # 144 (b,pixel) pairs per core
NQ = NPX * NH                     # 576 spectral problems per core
NG = NQ // 4                      # 144 block-diag groups of 4 problems
NBATCH = NG // 16                 # (unused)
NHB = NG // 8                     # 18 spectral half-batches (8 groups each)
SPEC_BF16 = True                  # spectral matmul dtype: bf16 vs fp32r
SPAT_BF16 = True                  # spatial matmul dtype: bf16 vs fp32r
MASK = 120.0                      # pre-scale mask magnitude (exp(-30) after scale)

TCW = [128, 128, 128, 128, 64]    # spatial t-chunks (sum 576)
PCW = [256, 256, 64]              # spatial p-chunks (sum 576)

_CACHE = {}


def _tf32_round(a):
    u = np.ascontiguousarray(a, dtype=np.float32).view(np.uint32).copy()
    lsb = (u >> 13) & np.uint32(1)
    u += np.uint32(0x0FFF) + lsb
    u &= np.uint32(0xFFFFE000)
    return u.view(np.float32)


def _spat_cast(a):
    if SPAT_BF16:
        import ml_dtypes
        return np.asarray(a, dtype=np.float32).astype(ml_dtypes.bfloat16)
    return _tf32_round(a)


# ---------------------------------------------------------------- host prep

def _prep_spatial(q2, kv2):
    """q2, kv2: [NS, HW, C] fp32 for this core's (b,s) slice.
    Returns kt [NS,16,NH,HW], qt [NS,16,NH,HW], vo [NS,5,128,NH*17]."""
    ns = q2.shape[0]
    # kt[i, d, h, t] = kv2[i, t, 16h+d]
    kt = np.ascontiguousarray(
        kv2.reshape(ns, HW, NH, HD).transpose(0, 3, 2, 1))
    qt = np.ascontiguousarray(
        q2.reshape(ns, HW, NH, HD).transpose(0, 3, 2, 1))
    vo = np.zeros((ns, 5, 128, NH * 17), np.float32)
    for tc in range(5):
        t0 = sum(TCW[:tc])
        tw = TCW[tc]
        blk = kv2[:, t0:t0 + tw, :].reshape(ns, tw, NH, HD)  # [ns,tw,h,d]
        for h in range(NH):
            vo[:, tc, :tw, 17 * h:17 * h + 16] = blk[:, :, h, :]
            vo[:, tc, :tw, 17 * h + 16] = 1.0
    return _spat_cast(kt), _spat_cast(qt), _spat_cast(vo)


def _prep_spectral(q1, kv1):
    """q1, kv1: [NPX, S, C] fp32 for this core's (b,px) slice.
    Problems q = px*4 + h; groups of 4 -> block-diag operands.
    Returns km [NG,21,128], qm [NG,21,128], vom [NG,128,NH*17]."""
    npx = q1.shape[0]
    # per problem: K/Q [S, HD]
    kq = kv1.reshape(npx, S, NH, HD).transpose(0, 2, 1, 3).reshape(NQ, S, HD)
    qq = q1.reshape(npx, S, NH, HD).transpose(0, 2, 1, 3).reshape(NQ, S, HD)
    km = np.zeros((NG, 21, 128), np.float32)
    qm = np.zeros((NG, 21, 128), np.float32)
    vom = np.zeros((NG, 128, 18), np.float32)
    for j in range(4):
        # lhsT rows 0..15: d, cols j*32+t = K[t, d]
        km[:, :16, j * 32:(j + 1) * 32] = kq[j::4].transpose(0, 2, 1)
        qm[:, :16, j * 32:(j + 1) * 32] = qq[j::4].transpose(0, 2, 1)
        # mask rows: scores^T[m=t-stack, n=s-stack] += sum_i lhsT[16+i,m]*rhs[16+i,n]
        km[:, 16 + j, j * 32:(j + 1) * 32] = MASK      # lhsT indicator * M
        qm[:, 16 + j, j * 32:(j + 1) * 32] = 1.0       # rhs indicator
        km[:, 20, :] = -MASK                           # constant -M row
        qm[:, 20, :] = 1.0
        vom[:, j * 32:(j + 1) * 32, :16] = kq[j::4]
        vom[:, j * 32:(j + 1) * 32, 16] = 1.0
    if SPEC_BF16:
        import ml_dtypes
        bf = ml_dtypes.bfloat16
        return km.astype(bf), qm.astype(bf), vom.astype(bf)
    return _tf32_round(km), _tf32_round(qm), _tf32_round(vom)


def _host_slices(query, feat):
    """Full inputs -> per-core input dicts (numpy)."""
    q4 = query.reshape(B * S, HW, C)        # (b,s) major
    f4 = feat.reshape(B * S, HW, C)
    q1 = query.transpose(0, 2, 3, 1, 4).reshape(B * HW, S, C)  # (b,px) major
    f1 = feat.transpose(0, 2, 3, 1, 4).reshape(B * HW, S, C)
    maps = []
    for c in range(NCORES):
        kt, qt, vo = _prep_spatial(q4[c * NS:(c + 1) * NS],
                                   f4[c * NS:(c + 1) * NS])
        km, qm, vom = _prep_spectral(q1[c * NPX:(c + 1) * NPX],
                                     f1[c * NPX:(c + 1) * NPX])
        maps.append(dict(kt=kt, qt=qt, vo=vo, km=km, qm=qm, vom=vom))
    return maps


def _decode(results):
    """Per-core outputs -> full [B,S,H,W,C]."""
    x_spat = np.zeros((B * S, HW, C), np.float32)
    x_spec = np.zeros((B * HW, S, C), np.float32)
    for c, r in enumerate(results):
        ospat = r["out_spat"]          # [NS, NH, 17, 576]
        for h in range(NH):
            num = ospat[:, h, :16, :]                      # [NS, 16, 576]
            den = ospat[:, h, 16:17, :]                    # [NS, 1, 576]
            x_spat[c * NS:(c + 1) * NS, :, 16 * h:16 * h + 16] = \
                (num / den).transpose(0, 2, 1)
        ospec = r["out_spec"]          # [NG, 128, 17]
        for j in range(4):
            # group g block j = (pixel g, head j)
            blk = ospec[:, j * 32:(j + 1) * 32, :]
            num = blk[:, :, :16]                            # [NG, 32, 16]
            den = blk[:, :, 16:17]
            x_spec[c * NPX:(c + 1) * NPX, :, 16 * j:16 * j + 16] = num / den
    full_spat = x_spat.reshape(B, S, HH, WW, C)
    full_spec = x_spec.reshape(B, HH, WW, S, C).transpose(0, 3, 1, 2, 4)
    return full_spat + full_spec


# ------------------------------------------------------------- device build

def _build_program():
    import concourse.bacc as bacc
    import concourse.tile as tile
    from concourse import mybir

    FP32 = mybir.dt.float32
    FP32R = mybir.dt.float32r
    SPEC_DT = mybir.dt.bfloat16 if SPEC_BF16 else FP32R
    SPAT_DT = mybir.dt.bfloat16 if SPAT_BF16 else FP32R
    EXP = mybir.ActivationFunctionType.Exp

    from concourse.bass import _add_dep_helper

    nc = bacc.Bacc("TRN2", target_bir_lowering=False, debug=False)

    d_kt = nc.dram_tensor("kt", [NS, 16, NH, HW], SPAT_DT, kind="ExternalInput")
    d_qt = nc.dram_tensor("qt", [NS, 16, NH, HW], SPAT_DT, kind="ExternalInput")
    d_vo = nc.dram_tensor("vo", [NS, 5, 128, NH * 17], SPAT_DT, kind="ExternalInput")
    SPEC_DT_D = mybir.dt.bfloat16 if SPEC_BF16 else FP32R
    d_km = nc.dram_tensor("km", [NG, 21, 128], SPEC_DT_D, kind="ExternalInput")
    d_qm = nc.dram_tensor("qm", [NG, 21, 128], SPEC_DT_D, kind="ExternalInput")
    d_vom = nc.dram_tensor("vom", [NG, 128, 18], SPEC_DT_D, kind="ExternalInput")
    d_ospat = nc.dram_tensor("out_spat", [NS, NH, 17, HW], FP32, kind="ExternalOutput")
    d_ospec = nc.dram_tensor("out_spec", [NG, 128, 17], FP32, kind="ExternalOutput")

    with tile.TileContext(nc) as tc:
        with (
            tc.tile_pool(name="cons", bufs=1) as cons,
            tc.tile_pool(name="io", bufs=3) as io,
            tc.tile_pool(name="spin", bufs=4) as spin,
            tc.tile_pool(name="pt", bufs=4) as ptp,
            tc.tile_pool(name="ps", bufs=1, space="PSUM") as ps,
        ):
            # persistent spatial V/ones operand
            vo_t = cons.tile([128, NS, 5, NH * 17], SPAT_DT)
            nc.sync.dma_start(vo_t[:], d_vo[:].rearrange("n tc p c -> p n tc c"))

            # all of PSUM as one tile; score sets = bank pairs {2s,2s+1},
            # s = chunk % 3; spatial PV accumulators live in banks 6,7.
            T = ps.tile([128, 8, 512], FP32, tag="T")

            kts, qts, sbos = {}, {}, {}
            kms, qms, voms = {}, {}, {}
            # explicit WAR guards: psum-region re-writers wait on the last
            # DVE evacuation copy of that region (bank-sharing races)
            guards = {}

            def guarded(inst, key):
                dep = guards.pop(key, None)
                if dep is not None:
                    _add_dep_helper(inst.ins, dep.ins, sync=True,
                                    reason=f"WAR evac {key}")
                return inst

            def load_bs(i):
                kts[i] = io.tile([128, NH, HW], SPAT_DT, tag="kt", name=f"kt{i}")
                qts[i] = io.tile([128, NH, HW], SPAT_DT, tag="qt", name=f"qt{i}")
                nc.sync.dma_start(kts[i][0:16], d_kt[i])
                nc.sync.dma_start(qts[i][0:16], d_qt[i])
                sbos[i] = io.tile([128, NH, HW], FP32, tag="so", name=f"so{i}")

            def load_hb(n):
                if n >= NHB:
                    return
                kms[n] = spin.tile([128, 8, 128], SPEC_DT, tag="km", name=f"km{n}")
                qms[n] = spin.tile([128, 8, 128], SPEC_DT, tag="qm", name=f"qm{n}")
                voms[n] = spin.tile([128, 8, 18], SPEC_DT, tag="vom", name=f"vom{n}")
                sl = np.s_[8 * n:8 * (n + 1)]
                nc.sync.dma_start(kms[n][0:21], d_km[sl].rearrange("g r c -> r g c"))
                nc.sync.dma_start(qms[n][0:21], d_qm[sl].rearrange("g r c -> r g c"))
                nc.sync.dma_start(voms[n][:], d_vom[sl].rearrange("g r c -> r g c"))

            def spat_front(n, item):
                _, i, pc, t_c = item
                if pc == 1 and t_c == 0 and i + 1 < NS:
                    load_bs(i + 1)
                p0 = sum(PCW[:pc]); pw = PCW[pc]
                t0 = sum(TCW[:t_c]); tw = TCW[t_c]
                s = n % 3
                for h in range(NH):
                    mm = nc.tensor.matmul(
                        T[0:tw, 2 * s + h // 2, 256 * (h % 2):256 * (h % 2) + pw],
                        kts[i][0:16, h, t0:t0 + tw],
                        qts[i][0:16, h, p0:p0 + pw],
                        start=True, stop=True, skip_group_check=True)
                    if h == 0:
                        guarded(mm, f"set{s}")

            def spat_back(n, item):
                _, i, pc, t_c = item
                p0 = sum(PCW[:pc]); pw = PCW[pc]
                t0 = sum(TCW[:t_c]); tw = TCW[t_c]
                s = n % 3
                pt_t = ptp.tile([128, 2, 512], SPAT_DT, tag="pt", name=f"pt{n}")
                nc.scalar.activation(
                    pt_t[0:tw].rearrange(
                        "p b (u c) -> p b u c", c=256)[:, :, :, 0:pw],
                    T[0:tw, 2 * s:2 * s + 2, :].rearrange(
                        "p b (u c) -> p b u c", c=256)[:, :, :, 0:pw],
                    EXP, scale=float(SCALE))
                first = {}
                for h in range(NH):
                    # start=True clears the WHOLE bank's has_written bits, so
                    # only the first head per bank may clear; the second
                    # head's tc==0 matmul overwrites (bits already clear).
                    mm = nc.tensor.matmul(
                        T[0:17, 6 + h // 2, 256 * (h % 2):256 * (h % 2) + pw],
                        vo_t[0:tw, i, t_c, 17 * h:17 * h + 17],
                        pt_t[0:tw, h // 2, 256 * (h % 2):256 * (h % 2) + pw],
                        start=(t_c == 0 and h % 2 == 0), stop=(t_c == 4),
                        skip_group_check=True)
                    if t_c == 0:
                        if h % 2 == 0:
                            first[h // 2] = mm
                            if h == 0:
                                guarded(mm, "pv")
                        else:
                            _add_dep_helper(mm.ins, first[h // 2].ins,
                                            sync=False,
                                            reason="bank clear order")
                if t_c == 4:
                    guards["pv"] = nc.vector.tensor_copy(
                        sbos[i][0:17, :, p0:p0 + pw],
                        T[0:17, 6:8, :].rearrange(
                            "p b (u c) -> p (b u) c", c=256)[:, :, 0:pw])
                    if pc == 2:
                        nc.sync.dma_start(
                            d_ospat[i].rearrange("h p c -> p h c"),
                            sbos[i][0:17])

            def spec_front(n, item):
                hb = item[1]
                load_hb(hb + 3)
                s = n % 3
                for g in range(8):
                    mm = nc.tensor.matmul(
                        T[:, 2 * s + g // 4, (g % 4) * 128:(g % 4) * 128 + 128],
                        kms[hb][0:21, g, :], qms[hb][0:21, g, :],
                        start=True, stop=True, skip_group_check=True)
                    if g == 0:
                        guarded(mm, f"set{s}")

            def spec_back(n, item):
                hb = item[1]
                s = n % 3
                pts = ptp.tile([128, 2, 512], SPEC_DT, tag="pt", name=f"pts{n}")
                nc.scalar.activation(pts[:], T[:, 2 * s:2 * s + 2, :],
                                     EXP, scale=float(SCALE))
                for g in range(8):
                    # 18-col pitch keeps fp32r psum dst 8B-aligned
                    nc.tensor.matmul(
                        T[:, 2 * s, 18 * g:18 * g + 18],
                        pts[:, g // 4, (g % 4) * 128:(g % 4) * 128 + 128],
                        voms[hb][:, g, :],
                        start=True, stop=True, skip_group_check=True)
                so_t = spin.tile([128, 8, 17], FP32, tag="spo", name=f"spo{n}")
                guards[f"set{s}"] = nc.vector.tensor_copy(
                    so_t[:], T[:, 2 * s, 0:144].rearrange(
                        "p (g c) -> p g c", c=18)[:, :, 0:17])
                nc.sync.dma_start(
                    d_ospec[8 * hb:8 * (hb + 1)].rearrange("g r c -> r g c"),
                    so_t[:])

            # ---- build interleaved chunk stream ----
            spat = [("spat", i, pc, t_c) for i in range(NS)
                    for pc in range(3) for t_c in range(5)]
            stream = []
            hb = 0
            for idx, itm in enumerate(spat):
                stream.append(itm)
                if hb < NHB and (idx + 1) * NHB >= (hb + 1) * len(spat):
                    stream.append(("spec", hb))
                    hb += 1
            while hb < NHB:
                stream.append(("spec", hb))
                hb += 1

            load_bs(0)
            for n in range(3):
                load_hb(n)

            fronts = {"spat": spat_front, "spec": spec_front}
            backs = {"spat": spat_back, "spec": spec_back}
            fronts[stream[0][0]](0, stream[0])
            for n in range(1, len(stream)):
                fronts[stream[n][0]](n, stream[n])
                backs[stream[n - 1][0]](n - 1, stream[n - 1])
            n = len(stream) - 1
            backs[stream[n][0]](n, stream[n])

    nc.compile()
    return nc


# ------------------------------------------------------------------ entry

def kernel(query, feat):
    from concourse.bass_utils import run_bass_kernel_spmd

    query = np.asarray(query, dtype=np.float32)
    feat = np.asarray(feat, dtype=np.float32)
    if "nc" not in _CACHE:
        _CACHE["nc"] = _build_program()
    nc = _CACHE["nc"]
    in_maps = _host_slices(query, feat)
    res = run_bass_kernel_spmd(nc, in_maps, core_ids=list(range(NCORES)))
    _CACHE["exec_time_ns"] = res.exec_time_ns
    _CACHE["res"] = res
    return _decode(res.results)


# ---------------------------------------------------- numpy device mirror

def _simulate_core(m):
    """Mirror of the device program in numpy, from prepped inputs to outputs."""
    out_spat = np.zeros((NS, NH, 17, HW), np.float32)
    kt = m["kt"].astype(np.float32)
    qt = m["qt"].astype(np.float32)
    vo = m["vo"].astype(np.float32)
    for i in range(NS):
        for h in range(NH):
            sT = kt[i, :, h, :].T @ qt[i, :, h, :]   # [t, p]
            pT = np.exp(SCALE * sT)
            acc = np.zeros((17, HW), np.float32)
            for t_c in range(5):
                t0 = sum(TCW[:t_c]); tw = TCW[t_c]
                acc += vo[i, t_c, :tw, 17 * h:17 * h + 17].T @ pT[t0:t0 + tw]
            out_spat[i, h] = acc
    out_spec = np.zeros((NG, 128, 17), np.float32)  # 17 useful cols
    for g in range(NG):
        sT = m["km"][g].astype(np.float32).T @ m["qm"][g].astype(np.float32)
        pT = np.exp(SCALE * sT)
        out_spec[g] = pT.T @ m["vom"][g].astype(np.float32)[:, :17]
    return dict(out_spat=out_spat, out_spec=out_spec)


def kernel_numpy(query, feat):
    """Host-only functional mirror (for layout validation)."""
    query = np.asarray(query, dtype=np.float32)
    feat = np.asarray(feat, dtype=np.float32)
    return _decode([_simulate_core(m) for m in _host_slices(query, feat)])

